# revision 3
# baseline (speedup 1.0000x reference)
"""Multi-head causal attention (B=1, S=4096, D=1024, H=16, HD=64) on 8
Trainium2 NeuronCores.

Sharding: head-parallel — 16 heads / 8 cores = 2 heads per core (one
128-channel slice of the QKV/output projections per core).

Per-core algorithm (all layouts transposed so the contraction dim sits on
SBUF partitions and softmax exp reads PSUM directly):
  phase 1  QKV projections from pre-transposed xT [D, S]:
             qT, kT [128, 4096] (d-contract matmuls, outputs transposed)
             V natural [4096, 128] via one extra PE transpose per 128-tile,
             stored interleaved with ones columns: [V_A | 1 | V_B | 1]
  phase 2  flash-style attention, no max-subtraction (scores ~ N(0,1)):
             scoresT psum [j, q] = kT_j.T @ qT_q  (2 heads packed via
             tile_position row strips, K=64 each)
             PT = exp(scoresT/8)  (ScalarE, reads PSUM, writes SBUF f32r)
             causal: strictly-upper j-blocks skipped, 4 diagonal mask
             tiles multiplied post-exp
             acc[65, q] += [V_j | 1].T @ PT_j  (M=65: row 64 = softmax
             denominator l for free)
             normalize: attnT[hd, q] = acc[0:64] * (1/l) (gpsimd
             partition-broadcast + DVE mul)
  phase 3  output projection partial: partialT[o, s] = WoT_c.T @ attnT,
             written transposed [1024, 4096] per core.

Host: sums the 8 partials and transposes back to [1, S, D].

Matmuls run in float32r (TF32-like, ~1.5e-4 rel err per matmul, 1 cyc/row
at N>=256 vs 4 cyc/row for plain fp32).
"""

import os
import sys

import numpy as np

for _p in ("/opt/trn_rl_repo", "/root/.axon_site/_ro/trn_rl_repo"):
    if os.path.isdir(_p) and _p not in sys.path:
        sys.path.insert(0, _p)

from contextlib import ExitStack

import concourse.bass as bass
import concourse.tile as tile
from concourse import bacc, bass_utils, mybir
from concourse.masks import make_identity

# Problem shape (hardcoded per the harness contract).
B, S, D, H = 1, 4096, 1024, 16
HD = D // H          # 64
NCORES = 8
HPC = H // NCORES    # 2 heads per core
M = HPC * HD         # 128 channels per core
SBK = 512            # s/q block size
NSB = S // SBK       # 8
DBK = 128            # d block size
NDB = D // DBK       # 8
JBK = 128            # j (key) block size
GJ = 3               # j-blocks per exp group ([128, 1536] psum = 3 banks)
VW = 2 * (HD + 1)    # v_aug row width per j-tile: [V_A | 1 | V_B | 1]

F32 = mybir.dt.float32
F32R = mybir.dt.float32r

_CACHE = {}


def _build_nc():
    """Build + compile the per-core Bass program (identical on all cores)."""
    nc = bacc.Bacc("TRN2", target_bir_lowering=False, debug=False,
                   num_devices=NCORES)

    xT = nc.dram_tensor("xT", [D, S], F32R, kind="ExternalInput").ap()
    wq = nc.dram_tensor("wq", [D, M], F32R, kind="ExternalInput").ap()
    wk = nc.dram_tensor("wk", [D, M], F32R, kind="ExternalInput").ap()
    wv = nc.dram_tensor("wv", [D, M], F32R, kind="ExternalInput").ap()
    wo = nc.dram_tensor("wo", [M, D], F32R, kind="ExternalInput").ap()
    dmask = nc.dram_tensor("dmask", [4, JBK, SBK], F32R,
                           kind="ExternalInput").ap()
    outp = nc.dram_tensor("outp", [D, S], F32, kind="ExternalOutput").ap()

    with tile.TileContext(nc) as tc:
        with ExitStack() as ctx:
            _emit(ctx, tc, nc, xT, wq, wk, wv, wo, dmask, outp)
    nc.compile()
    return nc


def _emit(ctx, tc, nc, xT, wq, wk, wv, wo, dmask, outp):
    const = ctx.enter_context(tc.tile_pool(name="const", bufs=1))
    persist = ctx.enter_context(tc.tile_pool(name="persist", bufs=1))
    xt_pool = ctx.enter_context(tc.tile_pool(name="xt", bufs=3))
    vtmp_pool = ctx.enter_context(tc.tile_pool(name="vtmp", bufs=2))
    pt_pool = ctx.enter_context(tc.tile_pool(name="pt", bufs=3))
    out_pool = ctx.enter_context(tc.tile_pool(name="outt", bufs=3))
    small = ctx.enter_context(tc.tile_pool(name="small", bufs=4))
    ps6k = ctx.enter_context(tc.tile_pool(name="ps6k", bufs=2, space="PSUM"))
    ps2k = ctx.enter_context(tc.tile_pool(name="ps2k", bufs=2, space="PSUM"))

    # ---- constants / persistent SBUF ----
    ident = const.tile([128, 128], F32)
    make_identity(nc, ident)

    wq_sb = const.tile([128, D], F32R)   # 8 d-tiles side by side [d, m]
    wk_sb = const.tile([128, D], F32R)
    wv_sb = const.tile([128, D], F32R)
    wo_sb = const.tile([128, D], F32R)   # [m, o]
    for d in range(NDB):
        nc.sync.dma_start(out=wq_sb[:, bass.ts(d, M)],
                          in_=wq[bass.ts(d, DBK), :])
        nc.sync.dma_start(out=wk_sb[:, bass.ts(d, M)],
                          in_=wk[bass.ts(d, DBK), :])
        nc.sync.dma_start(out=wv_sb[:, bass.ts(d, M)],
                          in_=wv[bass.ts(d, DBK), :])
    nc.sync.dma_start(out=wo_sb[:], in_=wo[:])

    mask_sb = const.tile([128, 4 * SBK], F32R)
    for r in range(4):
        nc.sync.dma_start(out=mask_sb[:, bass.ts(r, SBK)], in_=dmask[r])

    qT_sb = persist.tile([128, S], F32R)
    kT_sb = persist.tile([128, S], F32R)
    v_aug = persist.tile([128, (S // JBK) * VW], F32R)
    attnT = persist.tile([128, S], F32R)

    ones_sb = const.tile([128, 1], F32)
    nc.vector.memset(ones_sb[:], 1.0)

    def phase1(sb):
        """QKV projections for s-block sb (512 rows of the sequence)."""
        q_ps = ps6k.tile([128, SBK], F32, tag="sc")
        k_ps = ps6k.tile([128, SBK], F32, tag="sc")
        vT_ps = ps2k.tile([128, SBK], F32, tag="small")
        for d in range(NDB):
            xt = xt_pool.tile([128, SBK], F32R)
            nc.sync.dma_start(out=xt[:],
                              in_=xT[bass.ts(d, DBK), bass.ts(sb, SBK)])
            st, sp = d == 0, d == NDB - 1
            nc.tensor.matmul(q_ps[:], lhsT=wq_sb[:, bass.ts(d, M)],
                             rhs=xt[:], start=st, stop=sp)
            nc.tensor.matmul(k_ps[:], lhsT=wk_sb[:, bass.ts(d, M)],
                             rhs=xt[:], start=st, stop=sp)
            nc.tensor.matmul(vT_ps[:], lhsT=wv_sb[:, bass.ts(d, M)],
                             rhs=xt[:], start=st, stop=sp)
        nc.vector.tensor_copy(qT_sb[:, bass.ts(sb, SBK)], q_ps[:])
        nc.vector.tensor_copy(kT_sb[:, bass.ts(sb, SBK)], k_ps[:])
        vt = vtmp_pool.tile([128, SBK], F32)
        nc.vector.tensor_copy(vt[:], vT_ps[:])
        # vT [m, s] -> V natural [s, m] per 128-tile, into v_aug slots
        for t in range(SBK // JBK):
            jt = sb * (SBK // JBK) + t     # global j-tile index
            tp_ps = ps2k.tile([128, 128], F32, tag="small")
            nc.tensor.transpose(tp_ps[:], vt[:, bass.ts(t, 128)], ident[:])
            base = jt * VW
            nc.vector.tensor_copy(v_aug[:, base:base + HD],
                                  tp_ps[:, 0:HD])
            nc.vector.tensor_copy(v_aug[:, base + HD + 1:base + 2 * HD + 1],
                                  tp_ps[:, HD:2 * HD])
            nc.vector.tensor_copy(v_aug[:, base + HD:base + HD + 1],
                                  ones_sb[:])
            nc.vector.tensor_copy(v_aug[:, base + VW - 1:base + VW],
                                  ones_sb[:])

    def attention(qb):
        """Causal attention for query block qb (both heads)."""
        nj = 4 * (qb + 1)               # valid j128-blocks
        acc_A = ps2k.tile([HD + 1, SBK], F32, tag="small")
        acc_B = ps2k.tile([HD + 1, SBK], F32, tag="small")
        qsl = bass.ts(qb, SBK)
        for g in range((nj + GJ - 1) // GJ):
            jlo, jhi = GJ * g, min(GJ * (g + 1), nj)
            w = (jhi - jlo) * SBK
            sc_A = ps6k.tile([128, GJ * SBK], F32, tag="sc")
            sc_B = ps6k.tile([128, GJ * SBK], F32, tag="sc")
            for t, j in enumerate(range(jlo, jhi)):
                tsl = bass.ts(t, SBK)
                nc.tensor.matmul(sc_A[:, tsl],
                                 lhsT=kT_sb[0:64, bass.ts(j, JBK)],
                                 rhs=qT_sb[0:64, qsl],
                                 start=True, stop=True)
                nc.tensor.matmul(sc_B[:, tsl],
                                 lhsT=kT_sb[64:128, bass.ts(j, JBK)],
                                 rhs=qT_sb[64:128, qsl],
                                 start=True, stop=True)
            pt_A = pt_pool.tile([128, GJ * SBK], F32R, tag="pt")
            pt_B = pt_pool.tile([128, GJ * SBK], F32R, tag="pt")
            nc.scalar.activation(pt_A[:, 0:w], sc_A[:, 0:w],
                                 mybir.ActivationFunctionType.Exp,
                                 scale=float(1.0 / np.sqrt(HD)))
            nc.scalar.activation(pt_B[:, 0:w], sc_B[:, 0:w],
                                 mybir.ActivationFunctionType.Exp,
                                 scale=float(1.0 / np.sqrt(HD)))
            for t, j in enumerate(range(jlo, jhi)):
                tsl = bass.ts(t, SBK)
                r = j - (nj - 4)       # diagonal tile index (0..3) if >= 0
                if r >= 0:
                    msl = bass.ts(r, SBK)
                    nc.vector.tensor_mul(pt_A[:, tsl], pt_A[:, tsl],
                                         mask_sb[:, msl])
                    nc.vector.tensor_mul(pt_B[:, tsl], pt_B[:, tsl],
                                         mask_sb[:, msl])
                st, sp = j == 0, j == nj - 1
                vb = j * VW
                nc.tensor.matmul(acc_A[:], lhsT=v_aug[:, vb:vb + HD + 1],
                                 rhs=pt_A[:, tsl], start=st, stop=sp)
                nc.tensor.matmul(acc_B[:],
                                 lhsT=v_aug[:, vb + HD + 1:vb + VW],
                                 rhs=pt_B[:, tsl], start=st, stop=sp)
        # normalize: attnT rows = acc[0:64] / l  (l = acc row 64)
        for head, acc in ((0, acc_A), (1, acc_B)):
            linv = small.tile([1, SBK], F32, tag="linv")
            nc.vector.reciprocal(linv[:], acc[HD:HD + 1, :])
            linv64 = small.tile([64, SBK], F32, tag="linv64")
            nc.gpsimd.partition_broadcast(linv64[:], linv[:], channels=64)
            nc.vector.tensor_mul(attnT[head * 64:head * 64 + 64, qsl],
                                 acc[0:HD, :], linv64[:])

    def proj(qb):
        """Output-projection partial for s-block qb -> DRAM (transposed)."""
        qsl = bass.ts(qb, SBK)
        for ob in range(NDB):
            po = ps2k.tile([128, SBK], F32, tag="small")
            nc.tensor.matmul(po[:], lhsT=wo_sb[:, bass.ts(ob, 128)],
                             rhs=attnT[:, qsl], start=True, stop=True)
            ot = out_pool.tile([128, SBK], F32)
            nc.vector.tensor_copy(ot[:], po[:])
            nc.sync.dma_start(out=outp[bass.ts(ob, 128), qsl], in_=ot[:])

    # interleaved emission: attention(qb) only needs kT/v for s-blocks <= qb
    phase1(0)
    for sb in range(1, NSB):
        phase1(sb)
        attention(sb - 1)
        proj(sb - 1)
    attention(NSB - 1)
    proj(NSB - 1)


def _host_prep(x, Wq, Wk, Wv, Wo):
    xT = np.ascontiguousarray(x.reshape(S, D).T).astype(np.float32)
    dmask = np.zeros((4, JBK, SBK), dtype=np.float32)
    for r in range(4):
        jj = 128 * r + np.arange(JBK)[:, None]
        qq = np.arange(SBK)[None, :]
        dmask[r] = (jj <= qq).astype(np.float32)
    in_maps = []
    for c in range(NCORES):
        sl = slice(c * M, (c + 1) * M)
        in_maps.append({
            "xT": xT,
            "wq": np.ascontiguousarray(Wq[sl, :].T).astype(np.float32),
            "wk": np.ascontiguousarray(Wk[sl, :].T).astype(np.float32),
            "wv": np.ascontiguousarray(Wv[sl, :].T).astype(np.float32),
            "wo": np.ascontiguousarray(Wo[:, sl].T).astype(np.float32),
            "dmask": dmask,
        })
    return in_maps


def _run(inputs, trace=False):
    x = np.asarray(inputs["x"], dtype=np.float32)
    Wq = np.asarray(inputs["Wq"], dtype=np.float32)
    Wk = np.asarray(inputs["Wk"], dtype=np.float32)
    Wv = np.asarray(inputs["Wv"], dtype=np.float32)
    Wo = np.asarray(inputs["Wo"], dtype=np.float32)

    if "nc" not in _CACHE:
        _CACHE["nc"] = _build_nc()
    nc = _CACHE["nc"]

    in_maps = _host_prep(x, Wq, Wk, Wv, Wo)
    res = bass_utils.run_bass_kernel_spmd(
        nc, in_maps, core_ids=list(range(NCORES)), trace=trace)
    partial = np.zeros((D, S), dtype=np.float64)
    for c in range(NCORES):
        partial += res.results[c]["outp"].astype(np.float64)
    out = partial.T.astype(np.float32).reshape(B, S, D)
    return out, res


def kernel(x, mask, Wq, Wk, Wv, Wo):
    mask = np.asarray(mask)
    causal = np.tril(np.ones((S, S), dtype=bool))
    if mask.reshape(S, S).shape == causal.shape and bool(
            np.array_equal(mask.reshape(S, S), causal)):
        out, _ = _run({"x": x, "Wq": Wq, "Wk": Wk, "Wv": Wv, "Wo": Wo})
        return out
    # safety net for a non-causal mask: exact numpy fallback
    return _numpy_ref(np.asarray(x, np.float32), mask,
                      np.asarray(Wq, np.float32), np.asarray(Wk, np.float32),
                      np.asarray(Wv, np.float32), np.asarray(Wo, np.float32))


def _numpy_ref(x, mask, Wq, Wk, Wv, Wo):
    q = (x @ Wq.T).reshape(B, S, H, HD).transpose(0, 2, 1, 3)
    k = (x @ Wk.T).reshape(B, S, H, HD).transpose(0, 2, 1, 3)
    v = (x @ Wv.T).reshape(B, S, H, HD).transpose(0, 2, 1, 3)
    sc = np.einsum("bhqd,bhkd->bhqk", q, k) / np.sqrt(np.float32(HD))
    sc = np.where(mask.reshape(1, 1, S, S), sc, -1e9)
    sc = sc - sc.max(axis=-1, keepdims=True)
    p = np.exp(sc)
    p = p / p.sum(axis=-1, keepdims=True)
    o = np.einsum("bhqk,bhkd->bhqd", p, v)
    o = o.transpose(0, 2, 1, 3).reshape(B, S, D)
    return (o @ Wo.T).astype(np.float32)
